# revision 4
# baseline (speedup 1.0000x reference)
"""Trainium2 Bass kernel for a 7-layer ternary-weight (BitNet) 1D conv
feature extractor with exact-erf GELU after each layer.

Contract: kernel(**inputs) takes the FULL inputs from setup_inputs()
(x: [8, 160000] f32, w0..w6 / b0..b6 conv params) and returns the full
output [8, 256, 500] f32.

Strategy: data-parallel over batch — one batch element per NeuronCore,
8 cores. Weights are ternarized on host (sign in {-1,0,1} is exact in
fp16; the per-tensor absmean scale is folded into the GELU activation's
per-partition scale operand). Activations are fp16 in SBUF; every conv
is computed as K accumulating matmuls (one per tap, contraction = Cin
chunk) into fp32 PSUM, with stride-2 fp16 rhs access patterns so no
deinterleaving of intermediate activations is ever needed. Layer 0
(Cin=1, K=10, stride 5) uses a host-prepared 10-row phase buffer
xr[j, t] = xpad[5t + j] so its rhs is contiguous with contraction 10.
L0's input and output are streamed through SBUF ring buffers (with a
1-column halo on the L0 output) to fit the 208KB/partition budget;
layers 1-6 keep their outputs fully resident.
"""

import numpy as np

# (in_ch, out_ch, kernel, stride, pad) — fixed problem geometry
LAYERS = [(1, 128, 10, 5, 4), (128, 192, 3, 2, 1), (192, 192, 3, 2, 1),
          (192, 192, 3, 2, 1), (192, 256, 3, 2, 1), (256, 256, 4, 2, 1),
          (256, 256, 4, 2, 1)]
T_IN = 160000
LOUT = [32000, 16000, 8000, 4000, 2000, 1000, 500]
LIN = [T_IN] + LOUT[:-1]
N_CORES = 8
NT = 512        # matmul free-dim tile (one fp32 PSUM bank)
A0C = 8192      # L0-output chunk (ring buffered), multiple of 2*NT
XTC = 4096      # L0-input chunk (ring buffered), multiple of NT


def _chunks(c):
    return [(0, min(c, 128))] + ([(128, c - 128)] if c > 128 else [])


def _layout():
    """Column layout of the packed weight/bias tensors."""
    wcols = {}
    tot = 0
    for i, (cin, cout, k, s, p) in enumerate(LAYERS):
        groups = 1 if i == 0 else len(_chunks(cin)) * k
        wcols[i] = tot
        tot += groups * cout if i != 0 else cout
    nb = 0
    bcols = {}
    for i, (cin, cout, k, s, p) in enumerate(LAYERS):
        for mi, _ in enumerate(_chunks(cout)):
            bcols[(i, mi)] = nb
            nb += 2  # bias col + scale col
    return wcols, tot, bcols, nb


def _pack_host(ws, bs):
    """Ternarize weights; pack signs (fp16) and bias+scale (fp32)."""
    wcols, tot, bcols, nb = _layout()
    wpk = np.zeros((128, tot), np.float16)
    bpk = np.zeros((128, nb), np.float32)
    for i, (cin, cout, k, s, p) in enumerate(LAYERS):
        w = np.asarray(ws[i], np.float32)
        scale = max(float(np.mean(np.abs(w))), 1e-5)
        sign = np.clip(np.round(w / scale), -1.0, 1.0)  # [cout, cin, k]
        base = wcols[i]
        if i == 0:
            wpk[0:k, base:base + cout] = sign[:, 0, :].T.astype(np.float16)
        else:
            g = 0
            for (c0, csz) in _chunks(cin):
                for kk in range(k):
                    blk = sign[:, c0:c0 + csz, kk].T  # [csz, cout]
                    wpk[0:csz, base + g * cout: base + (g + 1) * cout] = \
                        blk.astype(np.float16)
                    g += 1
        b = np.asarray(bs[i], np.float32)
        for mi, (m0, msz) in enumerate(_chunks(cout)):
            c = bcols[(i, mi)]
            bpk[0:msz, c] = b[m0:m0 + msz]
            bpk[0:msz, c + 1] = scale
    return wpk, bpk


def _prep_x(xb):
    """Per-core L0 input: xr[j, t] = xpad[5t + j], xpad = 4-zero-padded x."""
    xpad = np.zeros(T_IN + 16, np.float16)
    xpad[4:4 + T_IN] = xb.astype(np.float16)
    L = LOUT[0]
    xr = np.empty((10, L), np.float16)
    for j in range(10):
        xr[j, :] = xpad[j:j + 5 * L:5]
    return xr


_CACHE = {}


def _build():
    """Build + compile the Bass program (weight-data-independent)."""
    if "nc" in _CACHE:
        return _CACHE["nc"]
    from concourse import bacc
    import concourse.mybir as mybir
    import concourse.tile as tile

    F16 = mybir.dt.float16
    F32 = mybir.dt.float32
    GELU = mybir.ActivationFunctionType.Gelu
    wcols, tot, bcols, nb = _layout()

    nc = bacc.Bacc("TRN2")
    xr_d = nc.dram_tensor("xr", [10, LOUT[0]], F16, kind="ExternalInput")
    wp_d = nc.dram_tensor("wp", [128, tot], F16, kind="ExternalInput")
    bp_d = nc.dram_tensor("bp", [128, nb], F32, kind="ExternalInput")
    y_d = nc.dram_tensor("y", [256, 500], F32, kind="ExternalOutput")

    with tile.TileContext(nc) as tc:
        pools = []

        def mkpool(name, bufs=1, space="SBUF"):
            p = tc.alloc_tile_pool(name=name, bufs=bufs, space=space)
            pools.append(p)
            return p

        wpool = mkpool("wpool")
        wt = wpool.tile([128, tot], F16, name="wt")
        bt = wpool.tile([128, nb], F32, name="bt")
        nc.sync.dma_start(out=wt[:, 0:128], in_=wp_d.ap()[:, 0:128])
        nc.sync.dma_start(out=bt[:, :], in_=bp_d.ap())
        nc.sync.dma_start(out=wt[:, 128:tot], in_=wp_d.ap()[:, 128:tot])

        opool = mkpool("opool")
        stage = [opool.tile([128, 500], F32, name=f"stage{m}")
                 for m in range(2)]
        pspool = mkpool("pspool", bufs=6, space="PSUM")
        xpool = mkpool("xpool", bufs=2)
        a0pool = mkpool("a0pool", bufs=2)

        # fully-resident output buffers for layers 1..5 (+ pad columns)
        act_tiles = {}
        for i in range(1, 6):
            cout = LAYERS[i][1]
            lout = LOUT[i]
            pool = mkpool(f"apool{i}")
            tiles = []
            for mi, (m0, msz) in enumerate(_chunks(cout)):
                t = pool.tile([msz, lout + 4], F16, name=f"a{i}_{mi}")
                nc.vector.memset(t[:, 0:1], 0.0)
                nc.vector.memset(t[:, lout + 1:lout + 3], 0.0)
                tiles.append(t)
            act_tiles[i] = tiles

        def emit_conv_tile(i, t0, n, rhs_of, dst_of):
            """One output tile [t0, t0+n) of layer i: matmuls + GELU.
            rhs_of(csz_idx, k) -> rhs AP;  dst_of(mi, msz) -> out AP."""
            cin, cout, k, s, p = LAYERS[i]
            cch = [(0, 10)] if i == 0 else _chunks(cin)
            for mi, (m0, msz) in enumerate(_chunks(cout)):
                ps = pspool.tile([msz, n], F32, name="ps", tag="ps")
                n_acc = len(cch) * (1 if i == 0 else k)
                a = 0
                for ci, (c0, csz) in enumerate(cch):
                    for kk in range(1 if i == 0 else k):
                        g = ci * k + kk
                        wb = (wcols[i] + m0 if i == 0
                              else wcols[i] + g * cout + m0)
                        lhsT = wt[0:csz, wb:wb + msz]
                        nc.tensor.matmul(ps[:, :], lhsT, rhs_of(ci, kk),
                                         start=(a == 0), stop=(a == n_acc - 1))
                        a += 1
                bc = bcols[(i, mi)]
                nc.scalar.activation(dst_of(mi, msz), ps[0:msz, 0:n], GELU,
                                     bias=bt[0:msz, bc:bc + 1],
                                     scale=bt[0:msz, bc + 1:bc + 2])

        # ---- layers 0+1 interleaved over A0C-sized chunks of L0 output ----
        # a0 chunk tile: col j holds L0-output u = cbase-1+j (col 0 = halo)
        n_ch = (LOUT[0] + A0C - 1) // A0C
        a0_tiles = [None] * n_ch

        def emit_l1(c, cbase, csz):
            src = a0_tiles[c]
            for t0 in range(cbase // 2, (cbase + csz) // 2, NT):
                n = min(NT, (cbase + csz) // 2 - t0)
                emit_conv_tile(
                    1, t0, n,
                    lambda ci, kk: src[0:128,
                                       2 * t0 + kk - cbase:
                                       2 * t0 + kk - cbase + 2 * n:2],
                    lambda mi, msz: act_tiles[1][mi][0:msz, 1 + t0:1 + t0 + n])

        for c in range(n_ch):
            cbase = c * A0C
            csz = min(A0C, LOUT[0] - cbase)
            at = a0pool.tile([128, A0C + 3], F16, tag="a0", name=f"a0_{c}")
            a0_tiles[c] = at
            if c == 0:
                nc.vector.memset(at[:, 0:1], 0.0)
            else:
                # left halo: duplicate previous chunk's last output column
                nc.vector.tensor_copy(at[:, 0:1],
                                      a0_tiles[c - 1][:, A0C:A0C + 1])
            for xb in range(cbase, cbase + csz, XTC):
                xn = min(XTC, LOUT[0] - xb)
                xt = xpool.tile([10, XTC], F16, tag="xt", name=f"xt_{xb}")
                nc.sync.dma_start(out=xt[:, 0:xn],
                                  in_=xr_d.ap()[:, xb:xb + xn])
                for t0 in range(xb, xb + xn, NT):
                    n = min(NT, xb + xn - t0)
                    emit_conv_tile(
                        0, t0, n,
                        lambda ci, kk: xt[0:10, t0 - xb:t0 - xb + n],
                        lambda mi, msz: at[0:msz,
                                           t0 - cbase + 1:t0 - cbase + 1 + n])
            if c > 0:
                emit_l1(c - 1, (c - 1) * A0C, A0C)
        emit_l1(n_ch - 1, (n_ch - 1) * A0C, csz)

        # ---- layers 2..6 on fully-resident buffers ----
        for i in range(2, 7):
            lout = LOUT[i]
            for t0 in range(0, lout, NT):
                n = min(NT, lout - t0)
                if i < 6:
                    def dst(mi, msz, i=i, t0=t0, n=n):
                        return act_tiles[i][mi][0:msz, 1 + t0:1 + t0 + n]
                else:
                    def dst(mi, msz, t0=t0, n=n):
                        return stage[mi][0:msz, t0:t0 + n]
                def rhs(ci, kk, i=i, t0=t0, n=n):
                    csz = _chunks(LAYERS[i][0])[ci][1]
                    col = 2 * t0 + kk
                    return act_tiles[i - 1][ci][0:csz, col:col + 2 * n:2]
                emit_conv_tile(i, t0, n, rhs, dst)

        nc.sync.dma_start(out=y_d.ap()[0:128, :], in_=stage[0][:, :])
        nc.sync.dma_start(out=y_d.ap()[128:256, :], in_=stage[1][:, :])
        for p in reversed(pools):
            p.release()

    nc.compile()
    _CACHE["nc"] = nc
    return nc


def kernel(x, w0, b0, w1, b1, w2, b2, w3, b3, w4, b4, w5, b5, w6, b6):
    import os
    from concourse.bass_utils import run_bass_kernel_spmd

    ws = [w0, w1, w2, w3, w4, w5, w6]
    bs = [b0, b1, b2, b3, b4, b5, b6]
    wpk, bpk = _pack_host(ws, bs)
    x = np.asarray(x, np.float32)
    in_maps = [{"xr": _prep_x(x[b]), "wp": wpk, "bp": bpk}
               for b in range(N_CORES)]
    nc = _build()
    trace = bool(os.environ.get("BITCONV_TRACE"))
    res = run_bass_kernel_spmd(nc, in_maps, core_ids=list(range(N_CORES)),
                               trace=trace)
    if trace:
        print(f"HW exec time: {res.exec_time_ns} ns")
        _CACHE["last_results"] = res
    return np.stack([res.results[b]["y"] for b in range(N_CORES)], axis=0)


# revision 6
# speedup vs baseline: 1.0316x; 1.0316x over previous
"""Trainium2 Bass kernel for a 7-layer ternary-weight (BitNet) 1D conv
feature extractor with exact-erf GELU after each layer.

Contract: kernel(**inputs) takes the FULL inputs from setup_inputs()
(x: [8, 160000] f32, w0..w6 / b0..b6 conv params) and returns the full
output [8, 256, 500] f32.

Strategy: data-parallel over batch — one batch element per NeuronCore,
8 cores. Weights are ternarized on host (sign in {-1,0,1} is exact in
fp16; the per-tensor absmean scale is folded into the GELU activation's
per-partition scale operand). Activations are fp16 in SBUF; every conv
is computed as K accumulating matmuls (one per tap, contraction = Cin
chunk) into fp32 PSUM, with stride-2 fp16 rhs access patterns so no
deinterleaving of intermediate activations is ever needed. Layer 0
(Cin=1, K=10, stride 5) uses a host-prepared 10-row phase buffer
xr[j, t] = xpad[5t + j] so its rhs is contiguous with contraction 10.
L0's input and output are streamed through SBUF ring buffers (with a
1-column halo on the L0 output) to fit the 208KB/partition budget;
layers 1-6 keep their outputs fully resident.
"""

import numpy as np

# (in_ch, out_ch, kernel, stride, pad) — fixed problem geometry
LAYERS = [(1, 128, 10, 5, 4), (128, 192, 3, 2, 1), (192, 192, 3, 2, 1),
          (192, 192, 3, 2, 1), (192, 256, 3, 2, 1), (256, 256, 4, 2, 1),
          (256, 256, 4, 2, 1)]
T_IN = 160000
LOUT = [32000, 16000, 8000, 4000, 2000, 1000, 500]
LIN = [T_IN] + LOUT[:-1]
N_CORES = 8
NT = 512        # matmul free-dim tile (one fp32 PSUM bank)
A0C = 8192      # L0-output chunk (ring buffered), multiple of 2*NT
XTC = 4096      # L0-input chunk (ring buffered), multiple of NT


def _chunks(c):
    return [(0, min(c, 128))] + ([(128, c - 128)] if c > 128 else [])


def _layout():
    """Column layout of the packed weight/bias tensors."""
    wcols = {}
    tot = 0
    for i, (cin, cout, k, s, p) in enumerate(LAYERS):
        groups = 1 if i == 0 else len(_chunks(cin)) * k
        wcols[i] = tot
        tot += groups * cout if i != 0 else cout
    nb = 0
    bcols = {}
    for i, (cin, cout, k, s, p) in enumerate(LAYERS):
        for mi, _ in enumerate(_chunks(cout)):
            bcols[(i, mi)] = nb
            nb += 2  # bias col + scale col
    return wcols, tot, bcols, nb


def _pack_host(ws, bs):
    """Ternarize weights; pack signs (fp16) and bias+scale (fp32)."""
    wcols, tot, bcols, nb = _layout()
    wpk = np.zeros((128, tot), np.float16)
    bpk = np.zeros((128, nb), np.float32)
    for i, (cin, cout, k, s, p) in enumerate(LAYERS):
        w = np.asarray(ws[i], np.float32)
        scale = max(float(np.mean(np.abs(w))), 1e-5)
        sign = np.clip(np.round(w / scale), -1.0, 1.0)  # [cout, cin, k]
        base = wcols[i]
        if i == 0:
            wpk[0:k, base:base + cout] = sign[:, 0, :].T.astype(np.float16)
        else:
            g = 0
            for (c0, csz) in _chunks(cin):
                for kk in range(k):
                    blk = sign[:, c0:c0 + csz, kk].T  # [csz, cout]
                    wpk[0:csz, base + g * cout: base + (g + 1) * cout] = \
                        blk.astype(np.float16)
                    g += 1
        b = np.asarray(bs[i], np.float32)
        for mi, (m0, msz) in enumerate(_chunks(cout)):
            c = bcols[(i, mi)]
            bpk[0:msz, c] = b[m0:m0 + msz]
            bpk[0:msz, c + 1] = scale
    return wpk, bpk


def _prep_x(xb):
    """Per-core L0 input: xr[j, t] = xpad[5t + j], xpad = 4-zero-padded x."""
    xpad = np.zeros(T_IN + 16, np.float16)
    xpad[4:4 + T_IN] = xb.astype(np.float16)
    L = LOUT[0]
    xr = np.empty((10, L), np.float16)
    for j in range(10):
        xr[j, :] = xpad[j:j + 5 * L:5]
    return xr


_CACHE = {}


def _build():
    """Build + compile the Bass program (weight-data-independent)."""
    if "nc" in _CACHE:
        return _CACHE["nc"]
    from concourse import bacc
    import concourse.mybir as mybir
    import concourse.tile as tile

    F16 = mybir.dt.float16
    F32 = mybir.dt.float32
    GELU = mybir.ActivationFunctionType.Gelu
    wcols, tot, bcols, nb = _layout()

    nc = bacc.Bacc("TRN2")
    xr_d = nc.dram_tensor("xr", [10, LOUT[0]], F16, kind="ExternalInput")
    wp_d = nc.dram_tensor("wp", [128, tot], F16, kind="ExternalInput")
    bp_d = nc.dram_tensor("bp", [128, nb], F32, kind="ExternalInput")
    y_d = nc.dram_tensor("y", [256, 500], F32, kind="ExternalOutput")

    with tile.TileContext(nc) as tc:
        pools = []

        def mkpool(name, bufs=1, space="SBUF"):
            p = tc.alloc_tile_pool(name=name, bufs=bufs, space=space)
            pools.append(p)
            return p

        wpool = mkpool("wpool")
        wt = wpool.tile([128, tot], F16, name="wt")
        bt = wpool.tile([128, nb], F32, name="bt")
        # L0 weights + biases only; the rest is DMAd after the first x
        # chunks so compute can start early.
        nc.sync.dma_start(out=wt[:, 0:128], in_=wp_d.ap()[:, 0:128])
        nc.sync.dma_start(out=bt[:, :], in_=bp_d.ap())

        opool = mkpool("opool")
        stage = [opool.tile([128, 500], F32, name=f"stage{m}")
                 for m in range(2)]
        pspool = mkpool("pspool", bufs=2, space="PSUM")
        xpool = mkpool("xpool", bufs=3)
        a0pool = mkpool("a0pool", bufs=2)

        # fully-resident output buffers for layers 1..5 (+ pad columns)
        act_tiles = {}
        for i in range(1, 6):
            cout = LAYERS[i][1]
            lout = LOUT[i]
            pool = mkpool(f"apool{i}")
            tiles = []
            for mi, (m0, msz) in enumerate(_chunks(cout)):
                t = pool.tile([msz, lout + 4], F16, name=f"a{i}_{mi}")
                nc.vector.memset(t[:, 0:1], 0.0)
                nc.vector.memset(t[:, lout + 1:lout + 3], 0.0)
                tiles.append(t)
            act_tiles[i] = tiles

        SNT = 4 * NT  # supertile: 4 PSUM banks accumulated, 1 GELU pass

        def emit_supertile(i, t0, nst, rhs_of, dst_of):
            """Output supertile [t0, t0+nst) of layer i: per 512-col slice
            an accumulating matmul group; one batched GELU over all banks.
            rhs_of(ci, kk, tt, n) -> rhs AP;  dst_of(mi, msz) -> out AP."""
            cin, cout, k, s, p = LAYERS[i]
            cch = [(0, 10)] if i == 0 else _chunks(cin)
            kr = 1 if i == 0 else k
            n_acc = len(cch) * kr
            for mi, (m0, msz) in enumerate(_chunks(cout)):
                ps = pspool.tile([msz, nst], F32, name="ps", tag="ps")
                for j0 in range(0, nst, NT):
                    n = min(NT, nst - j0)
                    a = 0
                    for ci, (c0, csz) in enumerate(cch):
                        for kk in range(kr):
                            g = ci * k + kk
                            wb = (wcols[i] + m0 if i == 0
                                  else wcols[i] + g * cout + m0)
                            lhsT = wt[0:csz, wb:wb + msz]
                            nc.tensor.matmul(
                                ps[:, j0:j0 + n], lhsT,
                                rhs_of(ci, kk, t0 + j0, n),
                                start=(a == 0), stop=(a == n_acc - 1))
                            a += 1
                bc = bcols[(i, mi)]
                nc.scalar.activation(dst_of(mi, msz), ps[0:msz, 0:nst], GELU,
                                     bias=bt[0:msz, bc:bc + 1],
                                     scale=bt[0:msz, bc + 1:bc + 2])

        # ---- layers 0+1 interleaved over A0C-sized chunks of L0 output ----
        # a0 chunk tile: col j holds L0-output u = cbase-1+j (col 0 = halo)
        n_ch = (LOUT[0] + A0C - 1) // A0C
        a0_tiles = [None] * n_ch

        def emit_l1(c, cbase, csz):
            src = a0_tiles[c]
            for t0 in range(cbase // 2, (cbase + csz) // 2, SNT):
                nst = min(SNT, (cbase + csz) // 2 - t0)
                emit_supertile(
                    1, t0, nst,
                    lambda ci, kk, tt, n: src[0:128,
                                              2 * tt + kk - cbase:
                                              2 * tt + kk - cbase + 2 * n:2],
                    lambda mi, msz, t0=t0, nst=nst:
                        act_tiles[1][mi][0:msz, 1 + t0:1 + t0 + nst])

        wrest = [False]

        def after_first_xt():
            if not wrest[0]:
                wrest[0] = True
                # L1 weight block, then everything else
                l1end = wcols[2]
                nc.sync.dma_start(out=wt[:, 128:l1end],
                                  in_=wp_d.ap()[:, 128:l1end])
                nc.sync.dma_start(out=wt[:, l1end:tot],
                                  in_=wp_d.ap()[:, l1end:tot])

        for c in range(n_ch):
            cbase = c * A0C
            csz = min(A0C, LOUT[0] - cbase)
            at = a0pool.tile([128, A0C + 3], F16, tag="a0", name=f"a0_{c}")
            a0_tiles[c] = at
            if c == 0:
                nc.vector.memset(at[:, 0:1], 0.0)
            else:
                # left halo: duplicate previous chunk's last output column
                nc.vector.tensor_copy(at[:, 0:1],
                                      a0_tiles[c - 1][:, A0C:A0C + 1])
            for xb in range(cbase, cbase + csz, XTC):
                xn = min(XTC, LOUT[0] - xb)
                xt = xpool.tile([10, XTC], F16, tag="xt", name=f"xt_{xb}")
                nc.sync.dma_start(out=xt[:, 0:xn],
                                  in_=xr_d.ap()[:, xb:xb + xn])
                after_first_xt()
                for t0 in range(xb, xb + xn, SNT):
                    nst = min(SNT, xb + xn - t0)
                    emit_supertile(
                        0, t0, nst,
                        lambda ci, kk, tt, n, xb=xb: xt[0:10, tt - xb:
                                                        tt - xb + n],
                        lambda mi, msz, t0=t0, nst=nst, cbase=cbase:
                            at[0:msz, t0 - cbase + 1:t0 - cbase + 1 + nst])
            if c > 0:
                emit_l1(c - 1, (c - 1) * A0C, A0C)
        emit_l1(n_ch - 1, (n_ch - 1) * A0C, csz)

        # ---- layers 2..6 on fully-resident buffers ----
        for i in range(2, 7):
            lout = LOUT[i]
            for t0 in range(0, lout, SNT):
                nst = min(SNT, lout - t0)
                if i < 6:
                    def dst(mi, msz, i=i, t0=t0, nst=nst):
                        return act_tiles[i][mi][0:msz, 1 + t0:1 + t0 + nst]
                else:
                    def dst(mi, msz, t0=t0, nst=nst):
                        return stage[mi][0:msz, t0:t0 + nst]
                def rhs(ci, kk, tt, n, i=i):
                    csz = _chunks(LAYERS[i][0])[ci][1]
                    col = 2 * tt + kk
                    return act_tiles[i - 1][ci][0:csz, col:col + 2 * n:2]
                emit_supertile(i, t0, nst, rhs, dst)

        nc.sync.dma_start(out=y_d.ap()[0:128, :], in_=stage[0][:, :])
        nc.sync.dma_start(out=y_d.ap()[128:256, :], in_=stage[1][:, :])
        for p in reversed(pools):
            p.release()

    nc.compile()
    _CACHE["nc"] = nc
    return nc


def kernel(x, w0, b0, w1, b1, w2, b2, w3, b3, w4, b4, w5, b5, w6, b6):
    import os
    from concourse.bass_utils import run_bass_kernel_spmd

    ws = [w0, w1, w2, w3, w4, w5, w6]
    bs = [b0, b1, b2, b3, b4, b5, b6]
    wpk, bpk = _pack_host(ws, bs)
    x = np.asarray(x, np.float32)
    in_maps = [{"xr": _prep_x(x[b]), "wp": wpk, "bp": bpk}
               for b in range(N_CORES)]
    nc = _build()
    trace = bool(os.environ.get("BITCONV_TRACE"))
    res = run_bass_kernel_spmd(nc, in_maps, core_ids=list(range(N_CORES)),
                               trace=trace)
    if trace:
        print(f"HW exec time: {res.exec_time_ns} ns")
        _CACHE["last_results"] = res
    return np.stack([res.results[b]["y"] for b in range(N_CORES)], axis=0)


# revision 7
# speedup vs baseline: 1.0362x; 1.0045x over previous
"""Trainium2 Bass kernel for a 7-layer ternary-weight (BitNet) 1D conv
feature extractor with exact-erf GELU after each layer.

Contract: kernel(**inputs) takes the FULL inputs from setup_inputs()
(x: [8, 160000] f32, w0..w6 / b0..b6 conv params) and returns the full
output [8, 256, 500] f32.

Strategy: data-parallel over batch — one batch element per NeuronCore,
8 cores. Weights are ternarized on host (sign in {-1,0,1} is exact in
fp16; the per-tensor absmean scale is folded into the GELU activation's
per-partition scale operand). Activations are fp16 in SBUF; every conv
is computed as K accumulating matmuls (one per tap, contraction = Cin
chunk) into fp32 PSUM, with stride-2 fp16 rhs access patterns so no
deinterleaving of intermediate activations is ever needed. Layer 0
(Cin=1, K=10, stride 5) uses a host-prepared 10-row phase buffer
xr[j, t] = xpad[5t + j] so its rhs is contiguous with contraction 10.
L0's input and output are streamed through SBUF ring buffers (with a
1-column halo on the L0 output) to fit the 208KB/partition budget;
layers 1-6 keep their outputs fully resident.
"""

import numpy as np

# (in_ch, out_ch, kernel, stride, pad) — fixed problem geometry
LAYERS = [(1, 128, 10, 5, 4), (128, 192, 3, 2, 1), (192, 192, 3, 2, 1),
          (192, 192, 3, 2, 1), (192, 256, 3, 2, 1), (256, 256, 4, 2, 1),
          (256, 256, 4, 2, 1)]
T_IN = 160000
LOUT = [32000, 16000, 8000, 4000, 2000, 1000, 500]
LIN = [T_IN] + LOUT[:-1]
N_CORES = 8
NT = 512        # matmul free-dim tile (one fp32 PSUM bank)
A0C = 8192      # L0-output chunk (ring buffered), multiple of 2*NT
XTC = 4096      # L0-input chunk (ring buffered), multiple of NT


def _chunks(c):
    return [(0, min(c, 128))] + ([(128, c - 128)] if c > 128 else [])


def _layout():
    """Column layout of the packed weight/bias tensors."""
    wcols = {}
    tot = 0
    for i, (cin, cout, k, s, p) in enumerate(LAYERS):
        groups = 1 if i == 0 else len(_chunks(cin)) * k
        wcols[i] = tot
        tot += groups * cout if i != 0 else cout
    nb = 0
    bcols = {}
    for i, (cin, cout, k, s, p) in enumerate(LAYERS):
        for mi, _ in enumerate(_chunks(cout)):
            bcols[(i, mi)] = nb
            nb += 2  # bias col + scale col
    return wcols, tot, bcols, nb


def _pack_host(ws, bs):
    """Ternarize weights; pack signs (fp16) and bias+scale (fp32)."""
    wcols, tot, bcols, nb = _layout()
    wpk = np.zeros((128, tot), np.float16)
    bpk = np.zeros((128, nb), np.float32)
    for i, (cin, cout, k, s, p) in enumerate(LAYERS):
        w = np.asarray(ws[i], np.float32)
        scale = max(float(np.mean(np.abs(w))), 1e-5)
        sign = np.clip(np.round(w / scale), -1.0, 1.0)  # [cout, cin, k]
        base = wcols[i]
        if i == 0:
            wpk[0:k, base:base + cout] = sign[:, 0, :].T.astype(np.float16)
        else:
            g = 0
            for (c0, csz) in _chunks(cin):
                for kk in range(k):
                    blk = sign[:, c0:c0 + csz, kk].T  # [csz, cout]
                    wpk[0:csz, base + g * cout: base + (g + 1) * cout] = \
                        blk.astype(np.float16)
                    g += 1
        b = np.asarray(bs[i], np.float32)
        for mi, (m0, msz) in enumerate(_chunks(cout)):
            c = bcols[(i, mi)]
            bpk[0:msz, c] = b[m0:m0 + msz]
            bpk[0:msz, c + 1] = scale
    return wpk, bpk


def _prep_x(xb):
    """Per-core L0 input: xr[j, t] = xpad[5t + j], xpad = 4-zero-padded x."""
    xpad = np.zeros(T_IN + 16, np.float16)
    xpad[4:4 + T_IN] = xb.astype(np.float16)
    L = LOUT[0]
    xr = np.empty((10, L), np.float16)
    for j in range(10):
        xr[j, :] = xpad[j:j + 5 * L:5]
    return xr


_CACHE = {}


def _build():
    """Build + compile the Bass program (weight-data-independent)."""
    if "nc" in _CACHE:
        return _CACHE["nc"]
    from concourse import bacc
    import concourse.mybir as mybir
    import concourse.tile as tile

    F16 = mybir.dt.float16
    F32 = mybir.dt.float32
    GELU = mybir.ActivationFunctionType.Gelu
    wcols, tot, bcols, nb = _layout()

    nc = bacc.Bacc("TRN2")
    xr_d = nc.dram_tensor("xr", [10, LOUT[0]], F16, kind="ExternalInput")
    wp_d = nc.dram_tensor("wp", [128, tot], F16, kind="ExternalInput")
    bp_d = nc.dram_tensor("bp", [128, nb], F32, kind="ExternalInput")
    y_d = nc.dram_tensor("y", [256, 500], F32, kind="ExternalOutput")

    with tile.TileContext(nc) as tc:
        pools = []

        def mkpool(name, bufs=1, space="SBUF"):
            p = tc.alloc_tile_pool(name=name, bufs=bufs, space=space)
            pools.append(p)
            return p

        wpool = mkpool("wpool")
        wt = wpool.tile([128, tot], F16, name="wt")
        bt = wpool.tile([128, nb], F32, name="bt")
        # L0 weights + biases only; the rest is DMAd after the first x
        # chunks so compute can start early.
        nc.sync.dma_start(out=wt[:, 0:128], in_=wp_d.ap()[:, 0:128])
        nc.sync.dma_start(out=bt[:, :], in_=bp_d.ap())

        opool = mkpool("opool")
        stage = [opool.tile([128, 500], F32, name=f"stage{m}")
                 for m in range(2)]
        pspool = mkpool("pspool", bufs=2, space="PSUM")
        xpool = mkpool("xpool", bufs=3)
        a0pool = mkpool("a0pool", bufs=2)

        # fully-resident output buffers for layers 1..5 (+ pad columns)
        act_tiles = {}
        for i in range(1, 6):
            cout = LAYERS[i][1]
            lout = LOUT[i]
            pool = mkpool(f"apool{i}")
            tiles = []
            for mi, (m0, msz) in enumerate(_chunks(cout)):
                t = pool.tile([msz, lout + 4], F16, name=f"a{i}_{mi}")
                nc.vector.memset(t[:, 0:1], 0.0)
                nc.vector.memset(t[:, lout + 1:lout + 3], 0.0)
                tiles.append(t)
            act_tiles[i] = tiles

        SNT = 4 * NT  # supertile: 4 PSUM banks accumulated, 1 GELU pass

        def emit_supertile(i, t0, nst, rhs_of, dst_of):
            """Output supertile [t0, t0+nst) of layer i: per 512-col slice
            an accumulating matmul group; one batched GELU over all banks.
            rhs_of(ci, kk, tt, n) -> rhs AP;  dst_of(mi, msz) -> out AP."""
            cin, cout, k, s, p = LAYERS[i]
            cch = [(0, 10)] if i == 0 else _chunks(cin)
            kr = 1 if i == 0 else k
            n_acc = len(cch) * kr
            for mi, (m0, msz) in enumerate(_chunks(cout)):
                ps = pspool.tile([msz, nst], F32, name="ps", tag="ps")
                for j0 in range(0, nst, NT):
                    n = min(NT, nst - j0)
                    a = 0
                    for ci, (c0, csz) in enumerate(cch):
                        for kk in range(kr):
                            g = ci * k + kk
                            wb = (wcols[i] + m0 if i == 0
                                  else wcols[i] + g * cout + m0)
                            lhsT = wt[0:csz, wb:wb + msz]
                            nc.tensor.matmul(
                                ps[:, j0:j0 + n], lhsT,
                                rhs_of(ci, kk, t0 + j0, n),
                                start=(a == 0), stop=(a == n_acc - 1))
                            a += 1
                bc = bcols[(i, mi)]
                nc.scalar.activation(dst_of(mi, msz), ps[0:msz, 0:nst], GELU,
                                     bias=bt[0:msz, bc:bc + 1],
                                     scale=bt[0:msz, bc + 1:bc + 2])

        # ---- layers 0+1 interleaved over A0C-sized chunks of L0 output ----
        # a0 chunk tile: col j holds L0-output u = cbase-1+j (col 0 = halo)
        n_ch = (LOUT[0] + A0C - 1) // A0C
        a0_tiles = [None] * n_ch

        def emit_l1(c, cbase, csz):
            src = a0_tiles[c]
            for t0 in range(cbase // 2, (cbase + csz) // 2, SNT):
                nst = min(SNT, (cbase + csz) // 2 - t0)
                emit_supertile(
                    1, t0, nst,
                    lambda ci, kk, tt, n: src[0:128,
                                              2 * tt + kk - cbase:
                                              2 * tt + kk - cbase + 2 * n:2],
                    lambda mi, msz, t0=t0, nst=nst:
                        act_tiles[1][mi][0:msz, 1 + t0:1 + t0 + nst])

        wrest = [0]

        def after_first_xt():
            # Bulk weight DMA goes via SWDGE (gpsimd) so it shares SDMA
            # round-robin with — instead of queuing ahead of — the
            # latency-critical x-chunk DMAs on the HWDGE path.
            if wrest[0] == 1:
                l1end = wcols[2]
                nc.gpsimd.dma_start(out=wt[:, 128:l1end],
                                    in_=wp_d.ap()[:, 128:l1end])
                nc.gpsimd.dma_start(out=wt[:, l1end:tot],
                                    in_=wp_d.ap()[:, l1end:tot])
            wrest[0] += 1

        for c in range(n_ch):
            cbase = c * A0C
            csz = min(A0C, LOUT[0] - cbase)
            at = a0pool.tile([128, A0C + 3], F16, tag="a0", name=f"a0_{c}")
            a0_tiles[c] = at
            if c == 0:
                nc.vector.memset(at[:, 0:1], 0.0)
            else:
                # left halo: duplicate previous chunk's last output column
                nc.vector.tensor_copy(at[:, 0:1],
                                      a0_tiles[c - 1][:, A0C:A0C + 1])
            for xb in range(cbase, cbase + csz, XTC):
                xn = min(XTC, LOUT[0] - xb)
                xt = xpool.tile([10, XTC], F16, tag="xt", name=f"xt_{xb}")
                nc.sync.dma_start(out=xt[:, 0:xn],
                                  in_=xr_d.ap()[:, xb:xb + xn])
                after_first_xt()
                for t0 in range(xb, xb + xn, SNT):
                    nst = min(SNT, xb + xn - t0)
                    emit_supertile(
                        0, t0, nst,
                        lambda ci, kk, tt, n, xb=xb: xt[0:10, tt - xb:
                                                        tt - xb + n],
                        lambda mi, msz, t0=t0, nst=nst, cbase=cbase:
                            at[0:msz, t0 - cbase + 1:t0 - cbase + 1 + nst])
            if c > 0:
                emit_l1(c - 1, (c - 1) * A0C, A0C)
        emit_l1(n_ch - 1, (n_ch - 1) * A0C, csz)

        # ---- layers 2..6 on fully-resident buffers ----
        for i in range(2, 7):
            lout = LOUT[i]
            for t0 in range(0, lout, SNT):
                nst = min(SNT, lout - t0)
                if i < 6:
                    def dst(mi, msz, i=i, t0=t0, nst=nst):
                        return act_tiles[i][mi][0:msz, 1 + t0:1 + t0 + nst]
                else:
                    def dst(mi, msz, t0=t0, nst=nst):
                        return stage[mi][0:msz, t0:t0 + nst]
                def rhs(ci, kk, tt, n, i=i):
                    csz = _chunks(LAYERS[i][0])[ci][1]
                    col = 2 * tt + kk
                    return act_tiles[i - 1][ci][0:csz, col:col + 2 * n:2]
                emit_supertile(i, t0, nst, rhs, dst)

        nc.sync.dma_start(out=y_d.ap()[0:128, :], in_=stage[0][:, :])
        nc.sync.dma_start(out=y_d.ap()[128:256, :], in_=stage[1][:, :])
        for p in reversed(pools):
            p.release()

    nc.compile()
    _CACHE["nc"] = nc
    return nc


def kernel(x, w0, b0, w1, b1, w2, b2, w3, b3, w4, b4, w5, b5, w6, b6):
    import os
    from concourse.bass_utils import run_bass_kernel_spmd

    ws = [w0, w1, w2, w3, w4, w5, w6]
    bs = [b0, b1, b2, b3, b4, b5, b6]
    wpk, bpk = _pack_host(ws, bs)
    x = np.asarray(x, np.float32)
    in_maps = [{"xr": _prep_x(x[b]), "wp": wpk, "bp": bpk}
               for b in range(N_CORES)]
    nc = _build()
    trace = bool(os.environ.get("BITCONV_TRACE"))
    res = run_bass_kernel_spmd(nc, in_maps, core_ids=list(range(N_CORES)),
                               trace=trace)
    if trace:
        print(f"HW exec time: {res.exec_time_ns} ns")
        _CACHE["last_results"] = res
    return np.stack([res.results[b]["y"] for b in range(N_CORES)], axis=0)
